# revision 11
# baseline (speedup 1.0000x reference)
"""Trainium2 Bass kernel: dense transformer block (B=2, T=2048, C=1024, H=16, HD=64).

Sharding over 8 NeuronCores: 2 batch groups (data parallel over B) x 4-way
tensor parallel within each group. Per group of 4 cores:
  - attention: heads split 4 ways (4 heads/core); per-core proj partials are
    ReduceScatter'd (bf16) over the token axis (each core receives a
    512-token shard of the summed attention output).
  - FFN: token-split (512 tokens/core), full W1/W2 streamed per core in bf16,
    in two 256-token halves so half 1 overlaps the tail attention chunk.
Each core returns its 512-token shard of the final output; the host
reassembles the full [2, 2048, 1024] tensor.

LayerNorm affine params are folded into the weight matrices host-side
(exactly): Wq/Wk/Wv scaled by g1 rows, W1 by g2 rows; beta contributions
become bias rows (bq/bk applied in the QKV PSUM->SBUF copy; beta1@Wv@Wp and
beta2@W1 folded into bp/b1).

Emission order is pipelined: per 512-token super-tile sc we emit LN1+
transpose+QKV+V (AB), then attention for q-chunk sc; LN2 vector work for
chunk ch is emitted as soon as its ReduceScatter is issued; FFN half 1
(tokens 0:256) is emitted before attention chunk 3 so it fills the tensor
engine while chunk 3's softmax exp runs on the scalar engine.
"""

import os
import sys

if "/opt/trn_rl_repo" not in sys.path:
    sys.path.insert(0, "/opt/trn_rl_repo")

import contextlib
import math

import ml_dtypes
import numpy as np

import concourse.bass as bass
import concourse.mybir as mybir
import concourse.tile as tile
from concourse import bacc
from concourse.bass_utils import run_bass_kernel_spmd
from concourse.masks import make_identity

# bass_utils' trace path imports antenv.axon_hooks, absent in this container.
# Register a graceful shim (and wire the boot-provided ctypes NTFF hook if
# available) so BASS_TRACE=1 profiles instead of crashing.
try:
    from antenv import axon_hooks as _ah  # noqa: F401
except ImportError:
    import types as _types

    _shim = _types.ModuleType("antenv.axon_hooks")
    _shim._hook = None
    _shim.set_axon_ntff_profile_hook = lambda h: setattr(_shim, "_hook", h)
    _shim.get_axon_ntff_profile_hook = lambda: _shim._hook
    sys.modules["antenv.axon_hooks"] = _shim
    try:
        if "/root/.axon_site" not in sys.path:
            sys.path.insert(0, "/root/.axon_site")
        from trn_agent_boot.trn_boot import _ntff_profile_via_ctypes

        _shim.set_axon_ntff_profile_hook(
            _ntff_profile_via_ctypes("/opt/axon/libaxon_pjrt.so")
        )
    except Exception:
        pass

AF = mybir.ActivationFunctionType
ALU = mybir.AluOpType
FP32 = mybir.dt.float32
BF16 = mybir.dt.bfloat16

P = 128
QCH = 512  # query chunk (free dim of S^T matmuls)
KG = 2  # k-tiles batched per exp() call


def build_block(T=2048, C=1024, NHL=4, F=4096, GC=4, eps=1e-5, n_cores=8):
    """Emit the per-core SPMD program. NHL = local heads (64-dim each)."""
    HD = 64
    DL = NHL * HD  # local head-dim total (256)
    NPAIR = NHL // 2
    NT = T // P  # token tiles (16)
    NCc = C // P  # channel tiles (8)
    NQC = T // QCH  # query chunks (4)
    KPC = QCH // P  # k-tiles per chunk (4)
    TSH = T // GC  # token shard (512)
    NST = TSH // P  # shard token tiles (4)
    NHT = F // P  # FFN hidden tiles (32)
    scale = 1.0 / math.sqrt(HD)
    HTOK = TSH // 2  # FFN half tokens (256)

    groups = [list(range(g * GC, (g + 1) * GC)) for g in range(n_cores // GC)]

    nc = bacc.Bacc(
        "TRN2", target_bir_lowering=False, num_devices=n_cores, debug=False
    )

    # ---- I/O ----
    x_full = nc.dram_tensor("x_full", [T, C], FP32, kind="ExternalInput")
    x_shard = nc.dram_tensor("x_shard", [TSH, C], FP32, kind="ExternalInput")
    wq_d = nc.dram_tensor("wq", [C, DL], BF16, kind="ExternalInput")
    wk_d = nc.dram_tensor("wk", [C, DL], BF16, kind="ExternalInput")
    wv_d = nc.dram_tensor("wv", [C, DL], BF16, kind="ExternalInput")
    wp_d = nc.dram_tensor("wp", [DL, C], BF16, kind="ExternalInput")
    w1_d = nc.dram_tensor("w1", [C, F], BF16, kind="ExternalInput")
    w2_d = nc.dram_tensor("w2", [F, C], BF16, kind="ExternalInput")
    b1_d = nc.dram_tensor("b1r", [P, NHT], FP32, kind="ExternalInput")
    b2_d = nc.dram_tensor("b2", [C], BF16, kind="ExternalInput")
    bqk_d = nc.dram_tensor("bqk", [P, 2 * NPAIR], FP32, kind="ExternalInput")
    mask_d = nc.dram_tensor("maskr", [P, P], BF16, kind="ExternalInput")
    out_d = nc.dram_tensor("out", [TSH, C], FP32, kind="ExternalOutput")

    with tile.TileContext(nc) as tc, contextlib.ExitStack() as est:
        big = est.enter_context(tc.tile_pool(name="big", bufs=NCc + 2 * NPAIR * 2))
        vp = est.enter_context(tc.tile_pool(name="vp", bufs=NT))
        h2Tp = est.enter_context(tc.tile_pool(name="h2Tp", bufs=NCc))
        x2p = est.enter_context(tc.tile_pool(name="x2p", bufs=NST))
        hid = est.enter_context(tc.tile_pool(name="hid", bufs=NHT))
        xt = est.enter_context(tc.tile_pool(name="xt", bufs=2))
        htp = est.enter_context(tc.tile_pool(name="htp", bufs=2))
        h2tp = est.enter_context(tc.tile_pool(name="h2tp", bufs=NST))
        rtp = est.enter_context(tc.tile_pool(name="rtp", bufs=1))
        stp = est.enter_context(tc.tile_pool(name="stp", bufs=3))
        exps = est.enter_context(tc.tile_pool(name="exps", bufs=3))
        pjp = est.enter_context(tc.tile_pool(name="pjp", bufs=4))
        rcp = est.enter_context(tc.tile_pool(name="rcp", bufs=2))
        rbc = est.enter_context(tc.tile_pool(name="rbc", bufs=2))
        w1p = est.enter_context(tc.tile_pool(name="w1p", bufs=2))
        w2p = est.enter_context(tc.tile_pool(name="w2p", bufs=3))
        obp = est.enter_context(tc.tile_pool(name="obp", bufs=2))
        sing = est.enter_context(tc.tile_pool(name="sing", bufs=1))
        psA = est.enter_context(tc.tile_pool(name="psA", bufs=2, space="PSUM"))
        psB = est.enter_context(tc.tile_pool(name="psB", bufs=2, space="PSUM"))
        psF = est.enter_context(tc.tile_pool(name="psF", bufs=2, space="PSUM"))
        dram = est.enter_context(tc.tile_pool(name="dram", bufs=1, space="DRAM"))

        # ---- singles ----
        ident = sing.tile([P, P], BF16, tag="ident", name="ident")
        make_identity(nc, ident)
        eps_t = sing.tile([P, 1], FP32, tag="eps", name="eps")
        nc.vector.memset(eps_t, eps)
        ones_t = sing.tile([1, HD], BF16, tag="ones", name="ones")
        nc.vector.memset(ones_t, 1.0)
        ones_col = sing.tile([P, 1], BF16, tag="onesc", name="onesc")
        nc.vector.memset(ones_col, 1.0)

        wq_sb = sing.tile([P, NCc, DL], BF16, tag="wq", name="wq")
        nc.sync.dma_start(wq_sb, wq_d.rearrange("(n p) m -> p n m", p=P))
        wk_sb = sing.tile([P, NCc, DL], BF16, tag="wk", name="wk")
        nc.sync.dma_start(wk_sb, wk_d.rearrange("(n p) m -> p n m", p=P))
        wv_sb = sing.tile([P, NCc, DL], BF16, tag="wv", name="wv")
        nc.sync.dma_start(wv_sb, wv_d.rearrange("(n p) m -> p n m", p=P))
        wp_sb = sing.tile([P, DL // P, C], BF16, tag="wp", name="wp")
        nc.sync.dma_start(wp_sb, wp_d.rearrange("(n p) m -> p n m", p=P))
        mask_sb = sing.tile([P, P], BF16, tag="mask", name="mask")
        nc.sync.dma_start(mask_sb, mask_d[:])
        b1_sb = sing.tile([P, NHT], FP32, tag="b1", name="b1")
        nc.sync.dma_start(b1_sb, b1_d[:])
        bqk_sb = sing.tile([P, 2 * NPAIR], FP32, tag="bqk", name="bqk")
        nc.sync.dma_start(bqk_sb, bqk_d[:])
        b2_bc = sing.tile([P, C], BF16, tag="b2bc", name="b2bc")
        b2_ap = b2_d[:]
        nc.sync.dma_start(
            b2_bc, bass.AP(tensor=b2_ap.tensor, offset=b2_ap.offset, ap=[[0, P]] + list(b2_ap.ap))
        )

        rs_in_t = [
            dram.tile([QCH, C], BF16, tag=f"rsin{k}", name=f"rsin{k}")
            for k in range(NQC)
        ]
        rs_out_t = [
            dram.tile([QCH // GC, C], BF16, tag=f"rsout{k}", name=f"rsout{k}")
            for k in range(NQC)
        ]

        # ---- LayerNorm stats helper (token-major [P, C] tile -> mu, rstd) ----
        fmax = math.gcd(512, C)
        nsub = C // fmax

        def ln_stats(x_t, tagp):
            stats = stp.tile([P, nsub, 6], FP32, tag=f"{tagp}_st", name=f"{tagp}_st")
            xr = x_t.rearrange("p (n f) -> p n f", n=nsub)
            for s in range(nsub):
                nc.vector.bn_stats(out=stats[:, s, :], in_=xr[:, s, :])
            mv = stp.tile([P, 2], FP32, tag=f"{tagp}_mv", name=f"{tagp}_mv")
            nc.vector.bn_aggr(out=mv, in_=stats)
            nc.scalar.activation(
                out=mv[:, 1:2], in_=mv[:, 1:2], func=AF.Sqrt, bias=eps_t, scale=1.0
            )
            nc.vector.reciprocal(out=mv[:, 1:2], in_=mv[:, 1:2])
            return mv

        # persistent big tiles
        hT = [big.tile([P, T], BF16, tag="big", name=f"hT{i}") for i in range(NCc)]
        QT = [big.tile([P, T], BF16, tag="big", name=f"QT{i}") for i in range(NPAIR)]
        KT = [big.tile([P, T], BF16, tag="big", name=f"KT{i}") for i in range(NPAIR)]
        V4 = [vp.tile([P, DL], BF16, tag="v", name=f"v{i}") for i in range(NT)]
        h2T = [h2Tp.tile([P, TSH], BF16, tag="h2T", name=f"h2T{i}") for i in range(NCc)]
        hidT = [hid.tile([P, TSH], BF16, tag="hid", name=f"hid{i}") for i in range(NHT)]
        x2pb = [None] * NST
        h2tiles = [None] * NST

        # ================= Phase AB: LN1 + transpose + QKV + V, per super-tile
        def emit_AB(sc):
            for it in range(4 * sc, 4 * sc + 4):
                x_t = xt.tile([P, C], FP32, tag="xb", name="xt")
                nc.sync.dma_start(x_t, x_full[it * P : (it + 1) * P, :])
                mv = ln_stats(x_t, "ln1")
                h_t = htp.tile([P, C], BF16, tag="hb", name="ht")
                nc.vector.tensor_scalar(
                    out=h_t,
                    in0=x_t,
                    scalar1=mv[:, 0:1],
                    scalar2=mv[:, 1:2],
                    op0=ALU.subtract,
                    op1=ALU.mult,
                )
                for ic in range(NCc):
                    ps = psF.tile([P, P], BF16, tag="psF", name="psF")
                    nc.tensor.transpose(ps, h_t[:, ic * P : (ic + 1) * P], ident)
                    dst = hT[ic][:, it * P : (it + 1) * P]
                    if (it + ic) % 2 == 0:
                        nc.vector.tensor_copy(dst, ps)
                    else:
                        nc.scalar.copy(dst, ps)
            # QKV for this chunk: feature-major head-pair stacked
            for pr in range(NPAIR):
                for qk, (dst_tiles, w_sb) in enumerate(((QT, wq_sb), (KT, wk_sb))):
                    ps = psB.tile([P, QCH], FP32, tag="psB", name="psB")
                    for ic in range(NCc):
                        for pos in range(2):
                            lh = 2 * pr + pos
                            nc.tensor.matmul(
                                ps[64 * pos : 64 * pos + 64, :],
                                lhsT=w_sb[:, ic, lh * HD : (lh + 1) * HD],
                                rhs=hT[ic][:, sc * QCH : (sc + 1) * QCH],
                                start=(ic == 0),
                                stop=(ic == NCc - 1),
                                tile_position=(0, 64 * pos),
                                skip_group_check=(pos == 1),
                            )
                    # copy + fold beta1@W bias row (exact; zero in practice)
                    nc.vector.tensor_scalar_add(
                        out=dst_tiles[pr][:, sc * QCH : (sc + 1) * QCH],
                        in0=ps,
                        scalar1=bqk_sb[:, qk * NPAIR + pr : qk * NPAIR + pr + 1],
                    )
            for it in range(4 * sc, 4 * sc + 4):
                ps = psB.tile([P, DL], FP32, tag="psB", name="psB")
                for ic in range(NCc):
                    nc.tensor.matmul(
                        ps,
                        lhsT=hT[ic][:, it * P : (it + 1) * P],
                        rhs=wv_sb[:, ic, :],
                        start=(ic == 0),
                        stop=(ic == NCc - 1),
                    )
                nc.vector.tensor_copy(V4[it], ps)

        # ================= Attention chunk ch + proj + ReduceScatter =========
        def emit_ATT(ch):
            n_kt = (ch + 1) * KPC
            n_kg = n_kt // KG
            attT = []
            for pr in range(NPAIR):
                att_ps = [
                    psB.tile([P, QCH], FP32, tag="psB", name="psB") for _ in range(2)
                ]

                def scores(kg):
                    s_ps = [
                        psA.tile([P, KG * QCH], FP32, tag="psA", name="psA")
                        for _ in range(2)
                    ]
                    for j in range(KG):
                        kt = kg * KG + j
                        for pos in range(2):
                            nc.tensor.matmul(
                                s_ps[pos][:, j * QCH : (j + 1) * QCH],
                                lhsT=KT[pr][
                                    64 * pos : 64 * pos + 64,
                                    kt * P : (kt + 1) * P,
                                ],
                                rhs=QT[pr][
                                    64 * pos : 64 * pos + 64,
                                    ch * QCH : (ch + 1) * QCH,
                                ],
                                start=True,
                                stop=True,
                                tile_position=(64 * pos, 0),
                            )
                    return s_ps

                s_cur = scores(0)
                for kg in range(n_kg):
                    # exp of current scores (scalar engine)
                    e_tiles = []
                    for pos in range(2):
                        e_sb = exps.tile([P, KG * QCH], BF16, tag="e", name="e")
                        nc.scalar.activation(
                            out=e_sb, in_=s_cur[pos], func=AF.Exp, scale=scale
                        )
                        e_tiles.append(e_sb)
                    # software pipeline: next kg's scores before this kg's AV
                    if kg + 1 < n_kg:
                        s_cur = scores(kg + 1)
                    # causal masking of diagonal blocks (vector engine)
                    for pos in range(2):
                        e_sb = e_tiles[pos]
                        for j in range(KG):
                            kt = kg * KG + j
                            jd = kt - ch * KPC
                            if 0 <= jd < KPC:
                                if jd > 0:
                                    nc.vector.memset(
                                        e_sb[:, j * QCH : j * QCH + jd * P], 0.0
                                    )
                                tri = slice(
                                    j * QCH + jd * P, j * QCH + (jd + 1) * P
                                )
                                nc.vector.tensor_mul(
                                    e_sb[:, tri], e_sb[:, tri], mask_sb
                                )
                    # AV + rowsum accumulation
                    for pos in range(2):
                        e_sb = e_tiles[pos]
                        for j in range(KG):
                            kt = kg * KG + j
                            lh = 2 * pr + pos
                            nc.tensor.matmul(
                                att_ps[pos][0:64, :],
                                lhsT=V4[kt][:, lh * HD : (lh + 1) * HD],
                                rhs=e_sb[:, j * QCH : (j + 1) * QCH],
                                start=(kt == 0),
                                stop=(kt == n_kt - 1),
                                tile_position=(0, 0),
                            )
                            nc.tensor.matmul(
                                att_ps[pos][64:65, :],
                                lhsT=ones_col,
                                rhs=e_sb[:, j * QCH : (j + 1) * QCH],
                                start=(kt == 0),
                                stop=(kt == n_kt - 1),
                                tile_position=(0, 64),
                                skip_group_check=True,
                            )
                # normalize: recip of [2, QCH] rowsums, gpsimd broadcast, mul
                at = pjp.tile([P, QCH], BF16, tag="attT", name="attT")
                bc_ps = psA.tile([P, KG * QCH], FP32, tag="psA", name="bcps")
                for pos in range(2):
                    rsum = rcp.tile([1, QCH], BF16, tag="rs2", name="rsum")
                    nc.scalar.copy(rsum, att_ps[pos][64:65, :])
                    nc.tensor.matmul(
                        bc_ps[64 * pos : 64 * pos + 64, 0:QCH],
                        lhsT=ones_t,
                        rhs=rsum,
                        start=True,
                        stop=True,
                        tile_position=(0, 64 * pos),
                    )
                rec_bc = rbc.tile([P, QCH], FP32, tag="rbc", name="rbc")
                nc.vector.reciprocal(out=rec_bc, in_=bc_ps[:, 0:QCH])
                for pos in range(2):
                    rows = slice(64 * pos, 64 * pos + 64)
                    nc.vector.tensor_mul(
                        at[rows, :], att_ps[pos][0:64, :], rec_bc[rows, :]
                    )
                attT.append(at)

            # proj partial (token-major, bf16) -> DMA straight to rs_in
            for tt in range(KPC):
                for oc in range(C // 512):
                    pj = psB.tile([P, 512], FP32, tag="psB", name="psB")
                    for pr in range(NPAIR):
                        nc.tensor.matmul(
                            pj,
                            lhsT=attT[pr][:, tt * P : (tt + 1) * P],
                            rhs=wp_sb[:, pr, oc * 512 : (oc + 1) * 512],
                            start=(pr == 0),
                            stop=(pr == NPAIR - 1),
                        )
                    pj_sb = pjp.tile([P, 512], BF16, tag="pj", name="pjsb")
                    if (tt + oc) % 2 == 0:
                        nc.vector.tensor_copy(pj_sb, pj)
                    else:
                        nc.scalar.copy(pj_sb, pj)
                    nc.sync.dma_start(
                        rs_in_t[ch][tt * P : (tt + 1) * P, oc * 512 : (oc + 1) * 512],
                        pj_sb,
                    )

            nc.gpsimd.collective_compute(
                "ReduceScatter",
                ALU.add,
                replica_groups=groups,
                ins=[rs_in_t[ch][:].opt()],
                outs=[rs_out_t[ch][:].opt()],
            )

        # ============ LN2 vector work for shard tile st (= chunk st) =========
        def emit_E_vec(st):
            r_t = rtp.tile([P, C], BF16, tag="rb", name="rt")
            nc.sync.dma_start(r_t, rs_out_t[st][:])
            xs_t = xt.tile([P, C], FP32, tag="xb", name="xst")
            nc.sync.dma_start(xs_t, x_shard[st * P : (st + 1) * P, :])
            x2_t = xt.tile([P, C], FP32, tag="xb", name="x2t")
            nc.vector.tensor_add(x2_t, xs_t, r_t)
            mv = ln_stats(x2_t, "ln2")
            h2_t = h2tp.tile([P, C], BF16, tag="h2b", name="h2t")
            nc.vector.tensor_scalar(
                out=h2_t,
                in0=x2_t,
                scalar1=mv[:, 0:1],
                scalar2=mv[:, 1:2],
                op0=ALU.subtract,
                op1=ALU.mult,
            )
            h2tiles[st] = h2_t
            xb = x2p.tile([P, C], BF16, tag="x2pb", name="x2pb")
            nc.vector.tensor_add(xb, x2_t, b2_bc)
            x2pb[st] = xb

        # transposes of LN2 output into h2T columns (tensor engine, batched)
        def emit_E_trans(st):
            h2_t = h2tiles[st]
            for ic in range(NCc):
                ps = psF.tile([P, P], BF16, tag="psF", name="psF")
                nc.tensor.transpose(ps, h2_t[:, ic * P : (ic + 1) * P], ident)
                dst = h2T[ic][:, st * P : (st + 1) * P]
                if (st + ic) % 2 == 0:
                    nc.vector.tensor_copy(dst, ps)
                else:
                    nc.scalar.copy(dst, ps)

        # ============ FFN halves =============================================
        def emit_W1(half):
            t0 = half * HTOK
            for h4 in range(NHT // 2):
                w1t = w1p.tile([P, NCc, 256], BF16, tag="w1t", name="w1t")
                nc.sync.dma_start(
                    w1t,
                    w1_d.rearrange("(n p) m -> p n m", p=P)[
                        :, :, h4 * 256 : (h4 + 1) * 256
                    ],
                )
                for hh in range(2):
                    ht = h4 * 2 + hh
                    hid_ps = psB.tile([P, HTOK], FP32, tag="psB", name="psB")
                    for ic in range(NCc):
                        nc.tensor.matmul(
                            hid_ps,
                            lhsT=w1t[:, ic, hh * P : (hh + 1) * P],
                            rhs=h2T[ic][:, t0 : t0 + HTOK],
                            start=(ic == 0),
                            stop=(ic == NCc - 1),
                        )
                    # bias + relu fused on vector: (hid + b1) max 0 -> bf16
                    nc.vector.tensor_scalar(
                        out=hidT[ht][:, t0 : t0 + HTOK],
                        in0=hid_ps,
                        scalar1=b1_sb[:, ht : ht + 1],
                        scalar2=0.0,
                        op0=ALU.add,
                        op1=ALU.max,
                    )

        def emit_W2(half):
            tts = (2 * half, 2 * half + 1)
            for oc in range(C // 512):
                grp = {
                    tt: psF.tile([P, 512], FP32, tag="psF", name="psF") for tt in tts
                }
                for ht in range(NHT):
                    w2t = w2p.tile([P, 512], BF16, tag="w2t", name="w2t")
                    nc.sync.dma_start(
                        w2t, w2_d[ht * P : (ht + 1) * P, oc * 512 : (oc + 1) * 512]
                    )
                    for tt in tts:
                        nc.tensor.matmul(
                            grp[tt],
                            lhsT=hidT[ht][:, tt * P : (tt + 1) * P],
                            rhs=w2t,
                            start=(ht == 0),
                            stop=(ht == NHT - 1),
                        )
                for tt in tts:
                    ob = obp.tile([P, 512], FP32, tag="ob", name="ob")
                    nc.vector.tensor_add(
                        ob, grp[tt], x2pb[tt][:, oc * 512 : (oc + 1) * 512]
                    )
                    nc.sync.dma_start(
                        out_d[tt * P : (tt + 1) * P, oc * 512 : (oc + 1) * 512], ob
                    )

        # ================= Emission schedule =================
        emit_AB(0)
        emit_ATT(0)
        emit_AB(1)
        emit_E_vec(0)
        emit_ATT(1)
        emit_AB(2)
        emit_E_vec(1)
        emit_ATT(2)
        emit_AB(3)
        emit_E_vec(2)
        emit_E_trans(0)
        emit_E_trans(1)
        emit_W1(0)
        emit_ATT(3)
        emit_E_vec(3)
        emit_W2(0)
        emit_E_trans(2)
        emit_E_trans(3)
        emit_W1(1)
        emit_W2(1)

    nc.finalize()
    return nc


# ------------------------- host side -------------------------

_CACHE = {}
LAST_RESULTS = None


def make_in_maps(inputs, T=2048, C=1024, H=16, F=4096, GC=4, n_cores=8):
    HD = 64
    NHL = H // GC
    DL = NHL * HD
    NHT = F // P
    bf = ml_dtypes.bfloat16

    x = np.asarray(inputs["x"], np.float32)
    Wq = np.asarray(inputs["Wq"], np.float32)
    Wk = np.asarray(inputs["Wk"], np.float32)
    Wv = np.asarray(inputs["Wv"], np.float32)
    Wp = np.asarray(inputs["Wp"], np.float32)
    bp = np.asarray(inputs["bp"], np.float32)
    W1 = np.asarray(inputs["W1"], np.float32)
    b1 = np.asarray(inputs["b1"], np.float32)
    W2 = np.asarray(inputs["W2"], np.float32)
    b2 = np.asarray(inputs["b2"], np.float32)
    g1 = np.asarray(inputs["g1"], np.float32)
    be1 = np.asarray(inputs["beta1"], np.float32)
    g2 = np.asarray(inputs["g2"], np.float32)
    be2 = np.asarray(inputs["beta2"], np.float32)

    # ---- exact host-side foldings of the LN affine params ----
    Wq_f = Wq * g1[None, :, None]  # [H, C, HD]
    Wk_f = Wk * g1[None, :, None]
    Wv_f = Wv * g1[None, :, None]
    bq = np.einsum("c,hcd->hd", be1, Wq)  # [H, HD]
    bk = np.einsum("c,hcd->hd", be1, Wk)
    bv = np.einsum("c,hcd->hd", be1, Wv)
    # softmax rows sum to 1 => V-bias passes through attention unchanged:
    # fold bv @ Wp (full, summed over all heads) into bp.
    bp_eff = bp + np.einsum("hd,hdc->c", bv, Wp.reshape(H, HD, C))
    W1_f = W1 * g2[:, None]
    b1_eff = b1 + be2 @ W1

    maskr = np.triu(np.ones((P, P), np.float32)).astype(bf)  # m[kr,qr]=kr<=qr
    b1r = np.ascontiguousarray(b1_eff.reshape(NHT, P).T)
    w1b = W1_f.astype(bf)
    w2b = W2.astype(bf)

    NQC = T // QCH
    RPC = QCH // GC  # shard rows per q-chunk (128)

    def shard_rows(g):
        return np.concatenate(
            [np.arange(k * QCH + g * RPC, k * QCH + (g + 1) * RPC) for k in range(NQC)]
        )

    NPAIR = NHL // 2
    in_maps = []
    for c in range(n_cores):
        b, g = c // GC, c % GC
        hsl = slice(g * NHL, (g + 1) * NHL)
        # bias rows for Q/K copies: [P, 2*NPAIR] (bq pairs then bk pairs)
        bq_loc = bq[hsl].reshape(NPAIR, P).T  # [128, NPAIR]
        bk_loc = bk[hsl].reshape(NPAIR, P).T
        bqk = np.ascontiguousarray(
            np.concatenate([bq_loc, bk_loc], axis=1).astype(np.float32)
        )
        in_maps.append(
            {
                "x_full": np.ascontiguousarray(x[b]),
                "x_shard": np.ascontiguousarray(x[b][shard_rows(g)] + bp_eff[None, :]),
                "wq": np.ascontiguousarray(
                    Wq_f[hsl].transpose(1, 0, 2).reshape(C, DL)
                ).astype(bf),
                "wk": np.ascontiguousarray(
                    Wk_f[hsl].transpose(1, 0, 2).reshape(C, DL)
                ).astype(bf),
                "wv": np.ascontiguousarray(
                    Wv_f[hsl].transpose(1, 0, 2).reshape(C, DL)
                ).astype(bf),
                "wp": np.ascontiguousarray(Wp[g * DL : (g + 1) * DL]).astype(bf),
                "w1": w1b,
                "w2": w2b,
                "b1r": b1r,
                "b2": b2.astype(bf),
                "bqk": bqk,
                "maskr": maskr,
            }
        )
    return in_maps


def kernel(**inputs) -> np.ndarray:
    global LAST_RESULTS
    B, T, C = inputs["x"].shape
    H = inputs["Wq"].shape[0]
    F = inputs["W1"].shape[1]
    GC = 4
    n_cores = 8
    key = (T, C, H, F)
    if key not in _CACHE:
        _CACHE[key] = build_block(T=T, C=C, NHL=H // GC, F=F, GC=GC, n_cores=n_cores)
    nc = _CACHE[key]
    in_maps = make_in_maps(inputs, T=T, C=C, H=H, F=F, GC=GC, n_cores=n_cores)
    res = run_bass_kernel_spmd(nc, in_maps, core_ids=list(range(n_cores)))
    LAST_RESULTS = res
    out = np.empty((B, T, C), np.float32)
    NQC = T // QCH
    RPC = QCH // GC
    for c in range(n_cores):
        b, g = c // GC, c % GC
        sh = res.results[c]["out"]
        for k in range(NQC):
            out[b, k * QCH + g * RPC : k * QCH + (g + 1) * RPC] = sh[
                k * RPC : (k + 1) * RPC
            ]
    return out
